# revision 34
# baseline (speedup 1.0000x reference)
"""IntersectionLoss Trainium2 kernel.

Math: loss_n = maskedmean_j relu(R + S*log(sum_i exp(-|t2_nj - t1_ni|^2/S) * m1_i + eps))
Key identity: |t2_j - t1_i|^2 = n2_j + n1_i - 2*t2_j.t1_i, so the inner sum is a
K=4 matmul G = t2aug^T.T @ t1aug^T with augmented rows
  t1aug = [x, y, z, -n1_i/2 + (S/2)ln m1_i],  t2aug = [x, y, z, 1]
(all bf16; n1/n2 are computed from the bf16-rounded coords so the pairwise
distance between rounded points is consistent).

PE: the 128x128 array is row-tiled into four independent 32x128 strips
(tile_position=(32q,0)); each plane's four 512-column bank matmuls run
concurrently in different strips (~4x effective row rate; the operands are
replicated into the four SBUF partition quadrants each strip reads).

The 8.4M-per-core pairwise exp+reduce is split across three engines
(ACT/DVE/GPS all saturate around 40-45us; ACT alone would need ~68us):
 - ACT planes (even): scalar.activation(Exp, scale=2/S, bias=-n2_j/S,
   accum_out) straight off the PSUM plane.
 - DVE planes (odd): Schraudolph exp bit-trick - tensor_scalar computes
   int16(round(G*SCH_A + schb_j)) whose BITS are the bf16 representation of
   exp(x) (bits = (z+127-c)*128, z = x*log2e, c chosen for zero mean error).
   The 2048-wide reduction then runs as: GPS tensor_tensor halving add
   2048->1024 (reading the int16 tile bitcast to bf16), a second halving
   1024->512 on GPS or DVE, and a 512-wide accumulate (tensor_scalar
   accum_out on DVE, or activation Identity accum_out on ACT) - placement
   chosen per plane by _dplane_plan to balance the three engines.
Pass-2 work is emitted several planes late (pending/gps_h2_q) so the
in-order DVE/GPS queues never stall on a cross-engine producer, and every
DVE plane owns private ei/yh/y2 tiles so the hot instructions carry at most
one semaphore wait (the HW queue structs fit only one).

Flash-style: the pairwise matrix only ever exists in PSUM (two 4-bank slots,
ping-pong). Sharding: data-parallel over N=16 across 8 cores (2 batches per
core). Final log/relu/masked-mean over (N,2048) runs on host in float64.

Measured on trn2: 73.3us HW exec vs 88.5us for the fp32r all-ACT baseline;
rel err ~2e-4 (tolerance 2e-2). Span anatomy: ~6us fixed semaphore-init
preamble + ~7us serial DMA issue/first plane + a ~2.9-3.2us/plane steady
state paced by the ACT chain (exp + matmul WAR + sem hops across the two
PSUM slots) with DVE/GPS work braided into the gaps.
"""

import sys

sys.path.insert(0, "/opt/trn_rl_repo")

import numpy as np
import ml_dtypes

import concourse.bass as bass
import concourse.tile as tile
from concourse import mybir
from concourse.bass_utils import run_bass_kernel_spmd

RADIUS = 1.0
SIGMA = 2.5
EPSILON = 1e-12

N, L1, L2 = 16, 2048, 2048
NCORES = 8
NB = N // NCORES  # batches per core
P = 128
A = L2 // P  # 16 j-tiles per batch
NPLANES = NB * A  # 32 planes per core
N_DVE = 15  # planes handled by the DVE Schraudolph path (of NPLANES)


def _dplane_plan(t):
    """Pass-2 placement for the t-th DVE plane: (h2 on GPS?, acc on ACT?).
    Spreads the halving/accumulate work so no single engine saturates
    (8 of 15 halvings on GPS, 5 of 15 accumulates on ACT)."""
    h2_gps = (t + 1) * 8 // 15 > t * 8 // 15
    acc_act = t % 3 == 1
    return h2_gps, acc_act

F32 = mybir.dt.float32
BF16 = mybir.dt.bfloat16
I16 = mybir.dt.int16
AF = mybir.ActivationFunctionType
OP = mybir.AluOpType

LOG2E = 1.4426950408889634
SCH_C = 0.0575327458840  # zero-mean Schraudolph shift
SCH_A = (2.0 / SIGMA) * LOG2E * 128.0  # scalar1 of DVE pass 1
SCH_B0 = (127.0 - SCH_C) * 128.0  # j-independent part of scalar2

_CACHE = {}


def _dve_planes():
    """Two ACT planes lead (their chain links land in the pipeline-fill
    phase where ACT idles anyway), then strict D/A alternation keeps both
    PSUM-evacuation engines fed from the two in-flight PSUM slots."""
    return {2 * t + 2 for t in range(N_DVE)}


def _acc_cols():
    """acc_sb columns, grouped by the engine that writes them so the final
    DRAM store can be two DMAs that each wait on a single producer engine.
    ACT planes take TWO columns (their exp is split into two 1024-wide halves
    whose partial sums the host adds)."""
    dve_set = _dve_planes()
    a_planes = [i for i in range(NPLANES) if i not in dve_set]
    d_planes = sorted(dve_set)
    accact_d = [i for t, i in enumerate(d_planes) if _dplane_plan(t)[1]]
    accdve_d = [i for t, i in enumerate(d_planes) if not _dplane_plan(t)[1]]
    col_of = {}
    c = 0
    for i in a_planes:
        col_of[i] = c
        c += 1
    for i in accact_d:
        col_of[i] = c
        c += 1
    n_actdma = c
    for i in accdve_d:
        col_of[i] = c
        c += 1
    return dve_set, col_of, n_actdma, c


NCOLS = NPLANES


def _build_program():
    nc = bass.Bass()
    # taug[k, (b s i)]: row k of the augmented operand, prepacked host-side
    taug_d = nc.declare_dram_parameter("taug", (4, NB * 2 * L1), BF16, isOutput=False)
    biasAV_d = nc.declare_dram_parameter("biasAV", (P, 2 * NPLANES), F32, isOutput=False)
    acc_d = nc.declare_dram_parameter("acc", (P, NCOLS), F32, isOutput=True)

    dve_set, col_of, n_act, ncols = _acc_cols()

    with tile.TileContext(nc) as tc:
        with (
            tc.tile_pool(name="consts", bufs=1) as consts,
            tc.tile_pool(name="sb", bufs=2) as sb,
            tc.tile_pool(name="ps", bufs=2, space="PSUM") as ps,
        ):
            # matmul operands replicated into the 4 SBUF partition quadrants
            # (row-tile Tq reads its operands from partitions 32q..32q+3).
            # DMA issue is serial on the Sync engine (~0.6us each), so order:
            # batch-0 operands (gate the first matmuls), biases (needed ~3us
            # later by the first consumers), then batch-1 operands.
            tT = consts.tile([128, NB * 2 * L1], BF16)
            biasAV = consts.tile([P, 2 * NPLANES], F32)
            biasA = biasAV[:, 0:NPLANES]
            biasV = biasAV[:, NPLANES : 2 * NPLANES]
            cs0 = slice(0, 2 * L1)
            nc.sync.dma_start(out=biasAV[:], in_=biasAV_d[:])
            for q in range(4):
                nc.sync.dma_start(out=tT[32 * q : 32 * q + 4, cs0], in_=taug_d[:, cs0])
            for b0 in range(1, NB):
                cs = slice(b0 * 2 * L1, (b0 + 1) * 2 * L1)
                for q in range(4):
                    nc.sync.dma_start(out=tT[32 * q : 32 * q + 4, cs], in_=taug_d[:, cs])
            # absorb the bias-DMA waits on their consumer engines so the hot
            # instructions carry <=1 wait each
            scrA = consts.tile([P, NPLANES], F32)
            nc.scalar.copy(scrA[:], biasA[:])
            scrV = consts.tile([P, 1], F32)
            nc.vector.tensor_scalar(
                out=scrV[:], in0=biasV[:, 0:1], scalar1=1.0, scalar2=None, op0=OP.mult
            )

            acc_sb = sb.tile([P, NCOLS], F32, tag="acc")
            escr_d = sb.tile([P, 512], BF16, tag="escr_d")
            escr_a = sb.tile([P, 512], BF16, tag="escr_a")

            # deferred second half of a DVE plane's reduction; emitted a couple
            # of dve planes later so no engine queue stalls waiting on a
            # cross-engine producer
            pending = []

            def flush_pending():
                e = pending.pop(0)
                y2 = e["y2"]
                if not e["y2_done"]:
                    nc.vector.tensor_tensor(
                        out=y2[:], in0=e["yh"][:, 0:512], in1=e["yh"][:, 512:1024], op=OP.add
                    )
                if e["acc_act"]:
                    nc.scalar.activation(
                        escr_a[:],
                        y2[:],
                        AF.Identity,
                        bias=0.0,
                        scale=1.0,
                        accum_out=acc_sb[:, e["col"]],
                    )
                else:
                    nc.vector.tensor_scalar(
                        out=escr_d[:],
                        in0=y2[:],
                        scalar1=1.0,
                        scalar2=0.0,
                        op0=OP.mult,
                        op1=OP.add,
                        accum_out=acc_sb[:, e["col"]],
                    )

            dve_t = 0
            gps_h2_q = []
            # one private tile per DVE plane: no buffer recycling, hence no
            # WAR semaphore waits on the hot instructions (the DVE/ACT queue
            # structs only fit one wait)
            ei_t = [consts.tile([P, L1], I16, name=f"ei{t}") for t in range(N_DVE)]
            yh_t = [consts.tile([P, 1024], BF16, name=f"yh{t}") for t in range(N_DVE)]
            y2_t = [consts.tile([P, 512], BF16, name=f"y2{t}") for t in range(N_DVE)]

            for b in range(NB):
                for jt in range(A):
                    idx = b * A + jt
                    g = ps.tile([P, L1], F32, tag="ps")
                    lhs_lo = (2 * b + 1) * L1 + jt * P
                    for it in range(L1 // 512):
                        qp = 32 * it
                        nc.tensor.matmul(
                            g[:, it * 512 : (it + 1) * 512],
                            tT[qp : qp + 4, lhs_lo : lhs_lo + P],
                            tT[
                                qp : qp + 4,
                                2 * b * L1 + it * 512 : 2 * b * L1 + (it + 1) * 512,
                            ],
                            start=True,
                            stop=True,
                            tile_position=(qp, 0),
                        )
                    col = slice(col_of[idx], col_of[idx] + 1)
                    bcol = slice(idx, idx + 1)
                    if idx in dve_set:
                        ei = ei_t[dve_t]
                        eb = ei[:].bitcast(BF16)
                        nc.vector.tensor_scalar(
                            out=ei[:],
                            in0=g[:],
                            scalar1=SCH_A,
                            scalar2=biasV[:, bcol],
                            op0=OP.mult,
                            op1=OP.add,
                        )
                        # reduce: GPS halves 2048->1024; then 1024->512 on GPS
                        # or DVE; then a 512-wide accumulate (ACT or DVE)
                        yh = yh_t[dve_t]
                        nc.gpsimd.tensor_tensor(
                            out=yh[:], in0=eb[:, 0:1024], in1=eb[:, 1024:2048], op=OP.add
                        )
                        h2_gps, acc_act = _dplane_plan(dve_t)
                        y2 = y2_t[dve_t]
                        dve_t += 1
                        if gps_h2_q:
                            e2 = gps_h2_q.pop(0)
                            nc.gpsimd.tensor_tensor(
                                out=e2["y2"][:],
                                in0=e2["yh"][:, 0:512],
                                in1=e2["yh"][:, 512:1024],
                                op=OP.add,
                            )
                        if len(pending) >= 4:
                            flush_pending()
                        entry = {"yh": yh, "col": col, "y2": y2, "y2_done": h2_gps, "acc_act": acc_act}
                        if h2_gps:
                            gps_h2_q.append(entry)
                        pending.append(entry)
                    else:
                        nc.scalar.activation(
                            g[:],
                            g[:],
                            AF.Exp,
                            bias=biasA[:, bcol],
                            scale=2.0 / SIGMA,
                            accum_out=acc_sb[:, col_of[idx] : col_of[idx] + 1],
                        )
                        if idx > 2 * N_DVE and pending:
                            flush_pending()
            while gps_h2_q:
                e2 = gps_h2_q.pop(0)
                nc.gpsimd.tensor_tensor(
                    out=e2["y2"][:],
                    in0=e2["yh"][:, 0:512],
                    in1=e2["yh"][:, 512:1024],
                    op=OP.add,
                )
            while pending:
                flush_pending()
            nc.sync.dma_start(out=acc_d[:, :n_act], in_=acc_sb[:, :n_act])
            nc.sync.dma_start(out=acc_d[:, n_act:], in_=acc_sb[:, n_act:])

    _elide_redundant_matmul_waits(nc)
    return nc


def _elide_redundant_matmul_waits(nc):
    """Drop semaphore waits on Matmult instrs that are transitively implied by
    their other waits (Tile emits per-proc-minimal, not transitively-minimal,
    waits; the PE Matmult queue struct only fits one sync wait command).

    Soundness: a wait (S, v) is removed only if chaining (a) same-engine
    in-order start/completion and (b) the completion vector clocks of the
    producers of the REMAINING waits already guarantees S >= v.
    """

    def merge(dst, src):
        for k, v in src.items():
            if dst.get(k, 0) < v:
                dst[k] = v

    all_insts = []
    for bb in nc.bb_map.values():
        all_insts.extend(bb.bb.instructions)
    if True:
        insts = all_insts
        n = len(insts)
        # cumulative updater ticks per semaphore
        sem_updaters = {}  # sem -> list of (cum_value, idx)
        sem_cum = {}
        idx_updates = [[] for _ in range(n)]  # idx -> [(sem, cum_after)]
        for idx, inst in enumerate(insts):
            si = inst.sync_info
            if not si:
                continue
            for u in si.on_update:
                s = u.ant_name
                v = getattr(u, "update_value", None) or 1
                c = sem_cum.get(s, 0) + v
                sem_cum[s] = c
                sem_updaters.setdefault(s, []).append((c, idx))
                idx_updates[idx].append((s, c))

        def producer_of(s, v):
            for c, uidx in sem_updaters.get(s, ()):
                if c >= v:
                    return uidx
            return None

        start_clock = [dict() for _ in range(n)]
        comp_clock = [dict() for _ in range(n)]
        for _ in range(3):
            prev_start = {}
            prev_comp = {}
            for idx, inst in enumerate(insts):
                e = str(inst.engine)
                sc = dict(prev_start.get(e, {}))
                si = inst.sync_info
                if si:
                    for w in si.on_wait:
                        s, v = w.ant_name, w.wait_value
                        if sc.get(s, 0) < v:
                            sc[s] = v
                        p = producer_of(s, v)
                        if p is not None:
                            merge(sc, comp_clock[p])
                cc = dict(sc)
                merge(cc, prev_comp.get(e, {}))
                for s, c in idx_updates[idx]:
                    if cc.get(s, 0) < c:
                        cc[s] = c
                start_clock[idx] = sc
                comp_clock[idx] = cc
                prev_start[e] = sc
                prev_comp[e] = cc

        # elide waits implied by remaining waits + engine order
        prev_start = {}
        eng_sem_cum = {}  # engine -> {sem: cumulative updates by this engine}
        for idx, inst in enumerate(insts):
            e = str(inst.engine)
            si = inst.sync_info
            if si and len(si.on_wait) > 1:
                waits = list(si.on_wait)
                kept = list(waits)
                # waits on semaphores produced by an EARLIER same-engine
                # instruction are implied by in-order engine execution
                own = eng_sem_cum.get(e, {})
                kept2 = [w for w in kept if own.get(w.ant_name, 0) < w.wait_value]
                if kept2:
                    kept = kept2
                for w in list(kept):
                    if len(kept) <= 1:
                        break
                    others = [x for x in kept if x is not w]
                    implied = dict(prev_start.get(e, {}))
                    for o in others:
                        if implied.get(o.ant_name, 0) < o.wait_value:
                            implied[o.ant_name] = o.wait_value
                        p = producer_of(o.ant_name, o.wait_value)
                        if p is not None:
                            merge(implied, comp_clock[p])
                    if implied.get(w.ant_name, 0) >= w.wait_value:
                        kept = others
                if len(kept) < len(waits):
                    si.on_wait = kept
                    inst.sync_info = si
            sc = dict(prev_start.get(e, {}))
            if si:
                for w in si.on_wait:
                    if sc.get(w.ant_name, 0) < w.wait_value:
                        sc[w.ant_name] = w.wait_value
                    p = producer_of(w.ant_name, w.wait_value)
                    if p is not None:
                        merge(sc, comp_clock[p])
            prev_start[e] = sc
            ec = eng_sem_cum.setdefault(e, {})
            for s, c in idx_updates[idx]:
                if ec.get(s, 0) < c:
                    ec[s] = c
    return nc


def _prep(t1, t2, mask1):
    """Build taug (N,4,2,L1) bf16 and the two bias arrays (N,P,A) f32.

    Coordinates are rounded to bf16 FIRST and n1/n2 computed from the rounded
    values, so the device-side |t2-t1|^2 reconstruction is consistent.
    """
    t1b = t1.astype(ml_dtypes.bfloat16)
    t2b = t2.astype(ml_dtypes.bfloat16)
    t1r = t1b.astype(np.float32)
    t2r = t2b.astype(np.float32)
    n1 = np.einsum("nik,nik->ni", t1r, t1r)  # (N, L1)
    n2 = np.einsum("njk,njk->nj", t2r, t2r)  # (N, L2)
    with np.errstate(divide="ignore"):
        w1 = -0.5 * n1 + (SIGMA / 2.0) * np.log(mask1)
    w1 = np.maximum(w1, -60.0)  # keep the Schraudolph int16 positive
    taug = np.empty((N, 4, 2, L1), ml_dtypes.bfloat16)
    taug[:, 0:3, 0, :] = t1b.transpose(0, 2, 1)
    taug[:, 3, 0, :] = w1.astype(ml_dtypes.bfloat16)
    taug[:, 0:3, 1, :] = t2b.transpose(0, 2, 1)
    taug[:, 3, 1, :] = 1.0
    # j = jt*128 + p  ->  bias[n, p, jt]
    biasA = (-n2 / SIGMA).reshape(N, A, P).transpose(0, 2, 1)
    biasV = (SCH_B0 - n2 * (128.0 * LOG2E / SIGMA)).reshape(N, A, P).transpose(0, 2, 1)
    return (
        taug,
        np.ascontiguousarray(biasA, np.float32),
        np.ascontiguousarray(biasV, np.float32),
    )


def _make_in_maps(t1, t2, mask1, mask2):
    t1 = np.asarray(t1, dtype=np.float32)
    t2 = np.asarray(t2, dtype=np.float32)
    mask1 = np.asarray(mask1, dtype=np.float32)
    taug, biasA, biasV = _prep(t1, t2, mask1)
    maps = []
    for c in range(NCORES):
        sl = slice(c * NB, (c + 1) * NB)
        tg = np.ascontiguousarray(
            taug[sl].transpose(1, 0, 2, 3).reshape(4, NB * 2 * L1)
        )
        maps.append(
            {
                "taug": tg,
                "biasAV": np.ascontiguousarray(
                    np.concatenate(
                        [
                            biasA[sl].transpose(1, 0, 2).reshape(P, NPLANES),
                            biasV[sl].transpose(1, 0, 2).reshape(P, NPLANES),
                        ],
                        axis=1,
                    )
                ),
            }
        )
    return maps


def kernel(t1, t2, mask1, mask2):
    if "nc" not in _CACHE:
        _CACHE["nc"] = _build_program()
    nc = _CACHE["nc"]

    in_maps = _make_in_maps(t1, t2, mask1, mask2)
    res = run_bass_kernel_spmd(nc, in_maps, list(range(NCORES)))

    # per core: acc[p, cols]; ACT planes hold two partial columns
    dve_set, col_of, _, _ = _acc_cols()
    acc = np.stack([r["acc"] for r in res.results]).astype(np.float64)  # (C, P, NCOLS)
    planes = np.empty((NCORES, P, NPLANES), np.float64)
    for idx in range(NPLANES):
        c = col_of[idx]
        planes[:, :, idx] = acc[:, :, c]
    acc_planes = planes.reshape(NCORES, P, NB, A)
    acc_full = acc_planes.transpose(0, 2, 3, 1).reshape(N, L2)

    d = RADIUS + SIGMA * np.log(acc_full + EPSILON)
    d = np.maximum(d, 0.0)
    m2 = np.asarray(mask2).astype(np.float64)
    loss = (d * m2).sum(axis=-1) / m2.sum(axis=-1)
    return loss.astype(np.float32)


# revision 36
# speedup vs baseline: 1.0053x; 1.0053x over previous
"""IntersectionLoss Trainium2 kernel.

Math: loss_n = maskedmean_j relu(R + S*log(sum_i exp(-|t2_nj - t1_ni|^2/S) * m1_i + eps))
Key identity: |t2_j - t1_i|^2 = n2_j + n1_i - 2*t2_j.t1_i, so the inner sum is a
K=4 matmul G = t2aug^T.T @ t1aug^T with augmented rows
  t1aug = [x, y, z, -n1_i/2 + (S/2)ln m1_i],  t2aug = [x, y, z, 1]
(all bf16; n1/n2 are computed from the bf16-rounded coords so the pairwise
distance between rounded points is consistent).

PE: the 128x128 array is row-tiled into four independent 32x128 strips
(tile_position=(32q,0)); each plane's four 512-column bank matmuls run
concurrently in different strips (~4x effective row rate; the operands are
replicated into the four SBUF partition quadrants each strip reads).

The 8.4M-per-core pairwise exp+reduce is split across three engines
(ACT/DVE/GPS all saturate around 40-45us; ACT alone would need ~68us):
 - ACT planes (even): scalar.activation(Exp, scale=2/S, bias=-n2_j/S,
   accum_out) straight off the PSUM plane.
 - DVE planes (odd): Schraudolph exp bit-trick - tensor_scalar computes
   int16(round(G*SCH_A + schb_j)) whose BITS are the bf16 representation of
   exp(x) (bits = (z+127-c)*128, z = x*log2e, c chosen for zero mean error).
   The 2048-wide reduction then runs as: GPS tensor_tensor halving add
   2048->1024 (reading the int16 tile bitcast to bf16), a second halving
   1024->512 on GPS or DVE, and a 512-wide accumulate (tensor_scalar
   accum_out on DVE, or activation Identity accum_out on ACT) - placement
   chosen per plane by _dplane_plan to balance the three engines.
Pass-2 work is emitted several planes late (pending/gps_h2_q) so the
in-order DVE/GPS queues never stall on a cross-engine producer, and every
DVE plane owns private ei/yh/y2 tiles so the hot instructions carry at most
one semaphore wait (the HW queue structs fit only one).

Flash-style: the pairwise matrix only ever exists in PSUM (two 4-bank slots,
ping-pong). Sharding: data-parallel over N=16 across 8 cores (2 batches per
core). Final log/relu/masked-mean over (N,2048) runs on host in float64.

Measured on trn2: 73.3us HW exec vs 88.5us for the fp32r all-ACT baseline;
rel err ~2e-4 (tolerance 2e-2). Span anatomy: ~6us fixed semaphore-init
preamble + ~7us serial DMA issue/first plane + a ~2.9-3.2us/plane steady
state paced by the ACT chain (exp + matmul WAR + sem hops across the two
PSUM slots) with DVE/GPS work braided into the gaps.
"""

import sys

sys.path.insert(0, "/opt/trn_rl_repo")

import numpy as np
import ml_dtypes

import concourse.bass as bass
import concourse.tile as tile
from concourse import mybir
from concourse.bass_utils import run_bass_kernel_spmd

RADIUS = 1.0
SIGMA = 2.5
EPSILON = 1e-12

N, L1, L2 = 16, 2048, 2048
NCORES = 8
NB = N // NCORES  # batches per core
P = 128
A = L2 // P  # 16 j-tiles per batch
NPLANES = NB * A  # 32 planes per core
N_DVE = 15  # planes handled by the DVE Schraudolph path (of NPLANES)


def _dplane_plan(t):
    """Pass-2 placement for the t-th DVE plane: (h2 on GPS?, acc on ACT?).
    Spreads the halving/accumulate work so no single engine saturates
    (8 of 15 halvings on GPS, 5 of 15 accumulates on ACT)."""
    h2_gps = (t + 1) * 8 // 15 > t * 8 // 15
    acc_act = t % 3 == 1
    return h2_gps, acc_act

F32 = mybir.dt.float32
BF16 = mybir.dt.bfloat16
I16 = mybir.dt.int16
AF = mybir.ActivationFunctionType
OP = mybir.AluOpType

LOG2E = 1.4426950408889634
SCH_C = 0.0575327458840  # zero-mean Schraudolph shift
SCH_A = (2.0 / SIGMA) * LOG2E * 128.0  # scalar1 of DVE pass 1
SCH_B0 = (127.0 - SCH_C) * 128.0  # j-independent part of scalar2

_CACHE = {}


def _dve_planes():
    """Two ACT planes lead (their chain links land in the pipeline-fill
    phase where ACT idles anyway), then strict D/A alternation keeps both
    PSUM-evacuation engines fed from the two in-flight PSUM slots."""
    return {2 * t + 2 for t in range(N_DVE)}


def _acc_cols():
    """acc_sb columns, grouped by the engine that writes them so the final
    DRAM store can be two DMAs that each wait on a single producer engine.
    ACT planes take TWO columns (their exp is split into two 1024-wide halves
    whose partial sums the host adds)."""
    dve_set = _dve_planes()
    a_planes = [i for i in range(NPLANES) if i not in dve_set]
    d_planes = sorted(dve_set)
    accact_d = [i for t, i in enumerate(d_planes) if _dplane_plan(t)[1]]
    accdve_d = [i for t, i in enumerate(d_planes) if not _dplane_plan(t)[1]]
    col_of = {}
    c = 0
    for i in a_planes:
        col_of[i] = c
        c += 1
    for i in accact_d:
        col_of[i] = c
        c += 1
    n_actdma = c
    for i in accdve_d:
        col_of[i] = c
        c += 1
    return dve_set, col_of, n_actdma, c


NCOLS = NPLANES


def _build_program():
    nc = bass.Bass()
    # taug[k, (b s i)]: row k of the augmented operand, prepacked host-side
    taug_d = nc.declare_dram_parameter("taug", (4, NB * 2 * L1), BF16, isOutput=False)
    biasAV_d = nc.declare_dram_parameter("biasAV", (P, 2 * NPLANES), F32, isOutput=False)
    acc_d = nc.declare_dram_parameter("acc", (P, NCOLS), F32, isOutput=True)

    dve_set, col_of, n_act, ncols = _acc_cols()

    with tile.TileContext(nc) as tc:
        with (
            tc.tile_pool(name="consts", bufs=1) as consts,
            tc.tile_pool(name="sb", bufs=2) as sb,
            tc.tile_pool(name="ps", bufs=2, space="PSUM") as ps,
        ):
            # matmul operands replicated into the 4 SBUF partition quadrants
            # (row-tile Tq reads its operands from partitions 32q..32q+3).
            # DMA issue is serial on the Sync engine (~0.6us each), so order:
            # batch-0 operands (gate the first matmuls), biases (needed ~3us
            # later by the first consumers), then batch-1 operands.
            tT = consts.tile([128, NB * 2 * L1], BF16)
            biasAV = consts.tile([P, 2 * NPLANES], F32)
            biasA = biasAV[:, 0:NPLANES]
            biasV = biasAV[:, NPLANES : 2 * NPLANES]
            cs0 = slice(0, 2 * L1)
            # tiny head DMA: just the two prefix ACT planes' bias columns, so
            # their exps aren't gated on the full bias transfer
            nc.sync.dma_start(out=biasAV[:, 0:2], in_=biasAV_d[:, 0:2])
            for q in range(4):
                nc.sync.dma_start(out=tT[32 * q : 32 * q + 4, cs0], in_=taug_d[:, cs0])
            nc.sync.dma_start(out=biasAV[:, 2:], in_=biasAV_d[:, 2:])
            for b0 in range(1, NB):
                cs = slice(b0 * 2 * L1, (b0 + 1) * 2 * L1)
                for q in range(4):
                    nc.sync.dma_start(out=tT[32 * q : 32 * q + 4, cs], in_=taug_d[:, cs])
            # absorb the bias-DMA waits on their consumer engines so the hot
            # instructions carry <=1 wait each
            scrA = consts.tile([P, NPLANES], F32)
            nc.scalar.copy(scrA[:, 0:2], biasA[:, 0:2])
            scrV = consts.tile([P, 1], F32)
            nc.vector.tensor_scalar(
                out=scrV[:], in0=biasV[:, 0:1], scalar1=1.0, scalar2=None, op0=OP.mult
            )

            acc_sb = sb.tile([P, NCOLS], F32, tag="acc")
            escr_d = sb.tile([P, 512], BF16, tag="escr_d")
            escr_a = sb.tile([P, 512], BF16, tag="escr_a")

            # deferred second half of a DVE plane's reduction; emitted a couple
            # of dve planes later so no engine queue stalls waiting on a
            # cross-engine producer
            pending = []

            def flush_pending():
                e = pending.pop(0)
                y2 = e["y2"]
                if not e["y2_done"]:
                    nc.vector.tensor_tensor(
                        out=y2[:], in0=e["yh"][:, 0:512], in1=e["yh"][:, 512:1024], op=OP.add
                    )
                if e["acc_act"]:
                    nc.scalar.activation(
                        escr_a[:],
                        y2[:],
                        AF.Identity,
                        bias=0.0,
                        scale=1.0,
                        accum_out=acc_sb[:, e["col"]],
                    )
                else:
                    nc.vector.tensor_scalar(
                        out=escr_d[:],
                        in0=y2[:],
                        scalar1=1.0,
                        scalar2=0.0,
                        op0=OP.mult,
                        op1=OP.add,
                        accum_out=acc_sb[:, e["col"]],
                    )

            dve_t = 0
            gps_h2_q = []
            # one private tile per DVE plane: no buffer recycling, hence no
            # WAR semaphore waits on the hot instructions (the DVE/ACT queue
            # structs only fit one wait)
            ei_t = [consts.tile([P, L1], I16, name=f"ei{t}") for t in range(N_DVE)]
            yh_t = [consts.tile([P, 1024], BF16, name=f"yh{t}") for t in range(N_DVE)]
            y2_t = [consts.tile([P, 512], BF16, name=f"y2{t}") for t in range(N_DVE)]

            for b in range(NB):
                for jt in range(A):
                    idx = b * A + jt
                    g = ps.tile([P, L1], F32, tag="ps")
                    lhs_lo = (2 * b + 1) * L1 + jt * P
                    for it in range(L1 // 512):
                        qp = 32 * it
                        nc.tensor.matmul(
                            g[:, it * 512 : (it + 1) * 512],
                            tT[qp : qp + 4, lhs_lo : lhs_lo + P],
                            tT[
                                qp : qp + 4,
                                2 * b * L1 + it * 512 : 2 * b * L1 + (it + 1) * 512,
                            ],
                            start=True,
                            stop=True,
                            tile_position=(qp, 0),
                        )
                    col = slice(col_of[idx], col_of[idx] + 1)
                    bcol = slice(idx, idx + 1)
                    if idx == 2:
                        nc.scalar.copy(scrA[:, 2:NPLANES], biasA[:, 2:NPLANES])
                    if idx in dve_set:
                        ei = ei_t[dve_t]
                        eb = ei[:].bitcast(BF16)
                        nc.vector.tensor_scalar(
                            out=ei[:],
                            in0=g[:],
                            scalar1=SCH_A,
                            scalar2=biasV[:, bcol],
                            op0=OP.mult,
                            op1=OP.add,
                        )
                        # reduce: GPS halves 2048->1024; then 1024->512 on GPS
                        # or DVE; then a 512-wide accumulate (ACT or DVE)
                        yh = yh_t[dve_t]
                        nc.gpsimd.tensor_tensor(
                            out=yh[:], in0=eb[:, 0:1024], in1=eb[:, 1024:2048], op=OP.add
                        )
                        h2_gps, acc_act = _dplane_plan(dve_t)
                        y2 = y2_t[dve_t]
                        dve_t += 1
                        if gps_h2_q:
                            e2 = gps_h2_q.pop(0)
                            nc.gpsimd.tensor_tensor(
                                out=e2["y2"][:],
                                in0=e2["yh"][:, 0:512],
                                in1=e2["yh"][:, 512:1024],
                                op=OP.add,
                            )
                        if len(pending) >= 4:
                            flush_pending()
                        entry = {"yh": yh, "col": col, "y2": y2, "y2_done": h2_gps, "acc_act": acc_act}
                        if h2_gps:
                            gps_h2_q.append(entry)
                        pending.append(entry)
                    else:
                        nc.scalar.activation(
                            g[:],
                            g[:],
                            AF.Exp,
                            bias=biasA[:, bcol],
                            scale=2.0 / SIGMA,
                            accum_out=acc_sb[:, col_of[idx] : col_of[idx] + 1],
                        )
                        if idx > 2 * N_DVE and pending:
                            flush_pending()
            while gps_h2_q:
                e2 = gps_h2_q.pop(0)
                nc.gpsimd.tensor_tensor(
                    out=e2["y2"][:],
                    in0=e2["yh"][:, 0:512],
                    in1=e2["yh"][:, 512:1024],
                    op=OP.add,
                )
            while pending:
                flush_pending()
            nc.sync.dma_start(out=acc_d[:, :n_act], in_=acc_sb[:, :n_act])
            nc.sync.dma_start(out=acc_d[:, n_act:], in_=acc_sb[:, n_act:])

    _elide_redundant_matmul_waits(nc)
    return nc


def _elide_redundant_matmul_waits(nc):
    """Drop semaphore waits on Matmult instrs that are transitively implied by
    their other waits (Tile emits per-proc-minimal, not transitively-minimal,
    waits; the PE Matmult queue struct only fits one sync wait command).

    Soundness: a wait (S, v) is removed only if chaining (a) same-engine
    in-order start/completion and (b) the completion vector clocks of the
    producers of the REMAINING waits already guarantees S >= v.
    """

    def merge(dst, src):
        for k, v in src.items():
            if dst.get(k, 0) < v:
                dst[k] = v

    all_insts = []
    for bb in nc.bb_map.values():
        all_insts.extend(bb.bb.instructions)
    if True:
        insts = all_insts
        n = len(insts)
        # cumulative updater ticks per semaphore
        sem_updaters = {}  # sem -> list of (cum_value, idx)
        sem_cum = {}
        idx_updates = [[] for _ in range(n)]  # idx -> [(sem, cum_after)]
        for idx, inst in enumerate(insts):
            si = inst.sync_info
            if not si:
                continue
            for u in si.on_update:
                s = u.ant_name
                v = getattr(u, "update_value", None) or 1
                c = sem_cum.get(s, 0) + v
                sem_cum[s] = c
                sem_updaters.setdefault(s, []).append((c, idx))
                idx_updates[idx].append((s, c))

        def producer_of(s, v):
            for c, uidx in sem_updaters.get(s, ()):
                if c >= v:
                    return uidx
            return None

        start_clock = [dict() for _ in range(n)]
        comp_clock = [dict() for _ in range(n)]
        for _ in range(3):
            prev_start = {}
            prev_comp = {}
            for idx, inst in enumerate(insts):
                e = str(inst.engine)
                sc = dict(prev_start.get(e, {}))
                si = inst.sync_info
                if si:
                    for w in si.on_wait:
                        s, v = w.ant_name, w.wait_value
                        if sc.get(s, 0) < v:
                            sc[s] = v
                        p = producer_of(s, v)
                        if p is not None:
                            merge(sc, comp_clock[p])
                cc = dict(sc)
                merge(cc, prev_comp.get(e, {}))
                for s, c in idx_updates[idx]:
                    if cc.get(s, 0) < c:
                        cc[s] = c
                start_clock[idx] = sc
                comp_clock[idx] = cc
                prev_start[e] = sc
                prev_comp[e] = cc

        # elide waits implied by remaining waits + engine order
        prev_start = {}
        eng_sem_cum = {}  # engine -> {sem: cumulative updates by this engine}
        for idx, inst in enumerate(insts):
            e = str(inst.engine)
            si = inst.sync_info
            if si and len(si.on_wait) > 1:
                waits = list(si.on_wait)
                kept = list(waits)
                # waits on semaphores produced by an EARLIER same-engine
                # instruction are implied by in-order engine execution
                own = eng_sem_cum.get(e, {})
                kept2 = [w for w in kept if own.get(w.ant_name, 0) < w.wait_value]
                if kept2:
                    kept = kept2
                for w in list(kept):
                    if len(kept) <= 1:
                        break
                    others = [x for x in kept if x is not w]
                    implied = dict(prev_start.get(e, {}))
                    for o in others:
                        if implied.get(o.ant_name, 0) < o.wait_value:
                            implied[o.ant_name] = o.wait_value
                        p = producer_of(o.ant_name, o.wait_value)
                        if p is not None:
                            merge(implied, comp_clock[p])
                    if implied.get(w.ant_name, 0) >= w.wait_value:
                        kept = others
                if len(kept) < len(waits):
                    si.on_wait = kept
                    inst.sync_info = si
            sc = dict(prev_start.get(e, {}))
            if si:
                for w in si.on_wait:
                    if sc.get(w.ant_name, 0) < w.wait_value:
                        sc[w.ant_name] = w.wait_value
                    p = producer_of(w.ant_name, w.wait_value)
                    if p is not None:
                        merge(sc, comp_clock[p])
            prev_start[e] = sc
            ec = eng_sem_cum.setdefault(e, {})
            for s, c in idx_updates[idx]:
                if ec.get(s, 0) < c:
                    ec[s] = c
    return nc


def _prep(t1, t2, mask1):
    """Build taug (N,4,2,L1) bf16 and the two bias arrays (N,P,A) f32.

    Coordinates are rounded to bf16 FIRST and n1/n2 computed from the rounded
    values, so the device-side |t2-t1|^2 reconstruction is consistent.
    """
    t1b = t1.astype(ml_dtypes.bfloat16)
    t2b = t2.astype(ml_dtypes.bfloat16)
    t1r = t1b.astype(np.float32)
    t2r = t2b.astype(np.float32)
    n1 = np.einsum("nik,nik->ni", t1r, t1r)  # (N, L1)
    n2 = np.einsum("njk,njk->nj", t2r, t2r)  # (N, L2)
    with np.errstate(divide="ignore"):
        w1 = -0.5 * n1 + (SIGMA / 2.0) * np.log(mask1)
    w1 = np.maximum(w1, -60.0)  # keep the Schraudolph int16 positive
    taug = np.empty((N, 4, 2, L1), ml_dtypes.bfloat16)
    taug[:, 0:3, 0, :] = t1b.transpose(0, 2, 1)
    taug[:, 3, 0, :] = w1.astype(ml_dtypes.bfloat16)
    taug[:, 0:3, 1, :] = t2b.transpose(0, 2, 1)
    taug[:, 3, 1, :] = 1.0
    # j = jt*128 + p  ->  bias[n, p, jt]
    biasA = (-n2 / SIGMA).reshape(N, A, P).transpose(0, 2, 1)
    biasV = (SCH_B0 - n2 * (128.0 * LOG2E / SIGMA)).reshape(N, A, P).transpose(0, 2, 1)
    return (
        taug,
        np.ascontiguousarray(biasA, np.float32),
        np.ascontiguousarray(biasV, np.float32),
    )


def _make_in_maps(t1, t2, mask1, mask2):
    t1 = np.asarray(t1, dtype=np.float32)
    t2 = np.asarray(t2, dtype=np.float32)
    mask1 = np.asarray(mask1, dtype=np.float32)
    taug, biasA, biasV = _prep(t1, t2, mask1)
    maps = []
    for c in range(NCORES):
        sl = slice(c * NB, (c + 1) * NB)
        tg = np.ascontiguousarray(
            taug[sl].transpose(1, 0, 2, 3).reshape(4, NB * 2 * L1)
        )
        maps.append(
            {
                "taug": tg,
                "biasAV": np.ascontiguousarray(
                    np.concatenate(
                        [
                            biasA[sl].transpose(1, 0, 2).reshape(P, NPLANES),
                            biasV[sl].transpose(1, 0, 2).reshape(P, NPLANES),
                        ],
                        axis=1,
                    )
                ),
            }
        )
    return maps


def kernel(t1, t2, mask1, mask2):
    if "nc" not in _CACHE:
        _CACHE["nc"] = _build_program()
    nc = _CACHE["nc"]

    in_maps = _make_in_maps(t1, t2, mask1, mask2)
    res = run_bass_kernel_spmd(nc, in_maps, list(range(NCORES)))

    # per core: acc[p, cols]; ACT planes hold two partial columns
    dve_set, col_of, _, _ = _acc_cols()
    acc = np.stack([r["acc"] for r in res.results]).astype(np.float64)  # (C, P, NCOLS)
    planes = np.empty((NCORES, P, NPLANES), np.float64)
    for idx in range(NPLANES):
        c = col_of[idx]
        planes[:, :, idx] = acc[:, :, c]
    acc_planes = planes.reshape(NCORES, P, NB, A)
    acc_full = acc_planes.transpose(0, 2, 3, 1).reshape(N, L2)

    d = RADIUS + SIGMA * np.log(acc_full + EPSILON)
    d = np.maximum(d, 0.0)
    m2 = np.asarray(mask2).astype(np.float64)
    loss = (d * m2).sum(axis=-1) / m2.sum(axis=-1)
    return loss.astype(np.float32)
